# revision 49
# baseline (speedup 1.0000x reference)
"""Mixtral GQA attention block (B=1, S=2048, HID=4096, NH=32, NKV=8, HD=128),
8-way tensor-parallel over heads on trn2: each core owns 4 query heads + 1 KV
head (one GQA group), computes its partial output projection, host sums the
8 partials.

Device layout notes:
  - All matmul operands are staged transposed (contraction dim on partitions).
    Host pre-packs every tensor partition-major so DMAs are identity copies.
  - The four projection GEMMs (q/k/v/o) run in fp8-e4m3 DoubleRow perf mode
    (2 contraction rows per PE pass) with 3-term hi/lo error compensation:
    W@x ~= Wh@xh + Wl@xh + Wh@xl where Wh=e4m3(W), Wl=e4m3(W-Wh). Operands
    are pre-scaled (h x16, weights x64, attn x16) to keep values and their
    fp8 residuals out of the e4m3 subnormal range; the 1/1024 unscale folds
    into existing PSUM-drain copies. Cuts projection PE time to 0.75x f16.
  - Attention itself stays f16/bf16: the score matmul contracts over HD=128
    (a single PE pass - DoubleRow can't help), and fp8 probabilities would
    flush small softmax rows to zero.
  - Scores are computed transposed (S^T[k,q]) so the exp'd tiles directly
    serve as lhsT for the P@V matmul; softmax denominator comes from an
    appended ones-column on V (memset 1/16 so the reciprocal yields 16/den
    and the output lands pre-scaled x16 for the fp8 split); no
    max-subtraction (bf16 probabilities cannot overflow; scores are O(20)).
  - Causality: only k-tiles j with j*128 <= q_max are computed; the
    diagonal-band blocks use a precomputed 0/1 multiplicative mask.
  - RoPE is applied in the transposed orientation; the half-swap crosses
    partitions so it runs as a PE matmul against a constant 128x128
    half-rotation permutation matrix.
"""

import math
import os
import sys

import numpy as np

sys.path.insert(0, "/opt/trn_rl_repo")

import concourse.bass as bass
import concourse.tile as tile
from concourse import bacc
from concourse import mybir

S = 2048
HID = 4096
NH, NKV, HD = 32, 8, 128
NCORES = 8
QH = NH // NCORES      # 4 query heads per core
ND = HID // 128        # 32 contraction chunks
NI = S // 512          # 4 q-chunks of 512
NJ = S // 128          # 16 k-tiles of 128
SCALE = 1.0 / math.sqrt(HD)
H_S = 16.0             # fp8 pre-scale on activations (h, attn)
W_S = 64.0             # fp8 pre-scale on weights
INV_HW = 1.0 / (H_S * W_S)

F16 = mybir.dt.float16
BF16 = mybir.dt.bfloat16
F32 = mybir.dt.float32
F8 = mybir.dt.float8e4
DR = mybir.MatmulPerfMode.DoubleRow

_CACHE = {}
LAST_RESULTS = None


def _build_program():
    nc = bacc.Bacc()

    # hi/lo fp8 pairs are stored in a leading [2] axis (0=hi, 1=lo)
    ht = nc.declare_dram_parameter("ht", [128, 2, ND, S], F8, isOutput=False)
    wq = nc.declare_dram_parameter("wq", [128, QH, 2, ND, 128], F8,
                                   isOutput=False)
    wk = nc.declare_dram_parameter("wk", [128, 2, ND, 128], F8, isOutput=False)
    wv = nc.declare_dram_parameter("wv", [128, 2, ND, 128], F8, isOutput=False)
    wo = nc.declare_dram_parameter("wo", [128, 2, QH, HID], F8, isOutput=False)
    cosd = nc.declare_dram_parameter("cosd", [128, S], F16, isOutput=False)
    pswap = nc.declare_dram_parameter("pswap", [128, 128], F16, isOutput=False)
    identd = nc.declare_dram_parameter("identd", [128, 128], F16,
                                       isOutput=False)
    identb = nc.declare_dram_parameter("identb", [128, 128], BF16,
                                       isOutput=False)
    maskd = nc.declare_dram_parameter("maskd", [128, 4, 512], BF16,
                                      isOutput=False)
    sind = nc.declare_dram_parameter("sind", [128, S], F16, isOutput=False)
    out = nc.declare_dram_parameter("out", [S, HID], F16, isOutput=True)

    with tile.TileContext(nc) as tc:
        with (
            tc.tile_pool(name="consts", bufs=1) as consts,
            tc.tile_pool(name="hpool", bufs=9) as hpool,
            tc.tile_pool(name="ptpool", bufs=5) as ptpool,
            tc.tile_pool(name="rtmp", bufs=3) as rtmp,
            tc.tile_pool(name="small", bufs=8) as small,
            tc.tile_pool(name="orow", bufs=5) as orowp,
            tc.tile_pool(name="pproj", bufs=1, space="PSUM") as pproj,
            tc.tile_pool(name="pwork", bufs=3, space="PSUM") as pwork,
            tc.tile_pool(name="popsum", bufs=4, space="PSUM") as popsum,
        ):
            # First pass activations + first head weights are the startup
            # critical path: enqueue exactly what the first matmuls touch
            # before any bulk weight traffic.
            wq_sb = consts.tile([128, QH, 2, ND, 128], F8)
            cos_sb = consts.tile([128, S], F16)
            sin_sb = consts.tile([128, S], F16)
            pswap_sb = consts.tile([128, 128], F16)
            wk_sb = consts.tile([128, 2, ND, 128], F8)
            wv_sb = consts.tile([128, 2, ND, 128], F8)
            wo_sb = consts.tile([128, 2, QH, HID], F8)
            ident16 = consts.tile([128, 128], F16)
            identbf = consts.tile([128, 128], BF16)
            masks = consts.tile([128, 4, 512], BF16)
            c1024 = consts.tile([128, 1], F32)
            nc.vector.memset(c1024, INV_HW)

            hts0 = []
            for dq in range(8):
                t_h = hpool.tile([128, 2, 4, 512], F8, tag="ht",
                                 name=f"ht_0_{dq}")
                hts0.append(t_h)

            def ht_dma(tiles, I, dq):
                nc.sync.dma_start(
                    out=tiles[dq],
                    in_=ht[:, :, dq * 4:(dq + 1) * 4, I * 512:(I + 1) * 512],
                )

            # Chunk 0 is DMA-bound: stream weights and h tiles in exact
            # first-use order (k/v/q0/q1 projections interleave per
            # d-pair), so the PE starts early and stays fed.
            for dq in range(8):
                d0 = dq * 4
                if dq == 0:
                    # first group at 4-slice granularity: the PE's first
                    # matmul only needs d0:d2 of each weight + h tile 0
                    for wsb, wdr in ((wk_sb, wk), (wv_sb, wv)):
                        nc.sync.dma_start(out=wsb[:, :, 0:4, :],
                                          in_=wdr[:, :, 0:4, :])
                    for tt in range(2):
                        nc.sync.dma_start(out=wq_sb[:, tt, :, 0:4, :],
                                          in_=wq[:, tt, :, 0:4, :])
                elif dq % 2 == 0:
                    dd = d0 - 4
                    nc.sync.dma_start(out=wk_sb[:, :, dd:dd + 8, :],
                                      in_=wk[:, :, dd:dd + 8, :])
                    nc.sync.dma_start(out=wv_sb[:, :, dd:dd + 8, :],
                                      in_=wv[:, :, dd:dd + 8, :])
                    nc.sync.dma_start(out=wq_sb[:, 0, :, dd:dd + 8, :],
                                      in_=wq[:, 0, :, dd:dd + 8, :])
                    nc.sync.dma_start(out=wq_sb[:, 1, :, dd:dd + 8, :],
                                      in_=wq[:, 1, :, dd:dd + 8, :])
                if dq == 2:
                    # rope tables for chunk 0 only (cols 0:512) + swap
                    # matrix: needed right as the k/q0 accumulators close
                    nc.sync.dma_start(out=pswap_sb, in_=pswap[:, :])
                    nc.sync.dma_start(out=cos_sb[:, 0:512],
                                      in_=cosd[:, 0:512])
                    nc.sync.dma_start(out=sin_sb[:, 0:512],
                                      in_=sind[:, 0:512])
                ht_dma(hts0, 0, dq)
            for wsb, wdr in ((wk_sb, wk), (wv_sb, wv)):
                nc.sync.dma_start(out=wsb[:, :, 28:32, :],
                                  in_=wdr[:, :, 28:32, :])
            for tt in range(2):
                nc.sync.dma_start(out=wq_sb[:, tt, :, 28:32, :],
                                  in_=wq[:, tt, :, 28:32, :])
            nc.sync.dma_start(out=identbf, in_=identb[:, :])
            nc.sync.dma_start(out=cos_sb[:, 512:], in_=cosd[:, 512:])
            nc.sync.dma_start(out=sin_sb[:, 512:], in_=sind[:, 512:])
            # 0/1 causal masks for the diagonal-band block offsets
            # (host-built): mask[m][p, f] = 1.0 iff f - p - 128*m >= 0
            nc.sync.dma_start(out=masks, in_=maskd[:, :, :])
            for tt in range(2, QH):
                nc.sync.dma_start(
                    out=wq_sb[:, tt, :, :, :],
                    in_=wq[:, tt, :, :, :],
                )
            nc.sync.dma_start(out=ident16, in_=identd[:, :])
            for oc in range(QH):
                for dh in range(2):
                    nc.sync.dma_start(
                        out=wo_sb[:, :, oc, dh * 2048:(dh + 1) * 2048],
                        in_=wo[:, :, oc, dh * 2048:(dh + 1) * 2048],
                    )

            qT = consts.tile([128, QH, S], F16)    # roped q, transposed
            kT = consts.tile([128, S], F16)        # roped k, transposed
            # V' tiles: per k-tile j, [128 tokens, 128 ch + 1/16-column]
            vA = consts.tile([128, NJ, 132], BF16)
            for j in range(NJ):
                nc.vector.memset(vA[:, j, 128:129], 1.0 / H_S)
            # attn out x16, transposed, fp8 hi/lo for the o-projection;
            # one tile per head-pair so the o-projection's first
            # accumulation pair doesn't wait on later heads' epilogues
            attnT8p = [consts.tile([128, 2, 2, S], F8, name=f"attnT8_{i}")
                       for i in range(2)]

            hts_by_chunk = {0: hts0}

            def do_proj(J):
                """Projection phase for chunk J: k/v/q0 interleaved per
                d-pair, then q1..q3, RoPE inline, v transposed into vA,
                next chunk's h tiles prefetched."""
                nsl = slice(J * 512, (J + 1) * 512)
                hts = hts_by_chunk[J]

                def proj3(targets):
                    # 3-term fp8 DoubleRow accumulation over the full HID
                    # contraction; wsel(sel, dp) -> lhsT [128, 2, M]
                    for dp in range(ND // 2):
                        d0 = 2 * dp
                        dq, loc = d0 // 4, d0 % 4
                        rh = hts[dq][:, 0, loc:loc + 2, :]
                        rl = hts[dq][:, 1, loc:loc + 2, :]
                        for ps, wsel in targets:
                            nc.tensor.matmul(ps, wsel(0, dp), rh,
                                             start=(dp == 0), stop=False,
                                             perf_mode=DR)
                            nc.tensor.matmul(ps, wsel(1, dp), rh,
                                             start=False, stop=False,
                                             perf_mode=DR)
                            nc.tensor.matmul(ps, wsel(0, dp), rl,
                                             start=False,
                                             stop=(dp == ND // 2 - 1),
                                             perf_mode=DR)

                def rope_into(ps, dst, width=512):
                    """ps: PSUM [128, width] f32 pre-rope x1024 (transposed
                    layout). dst: f16 SBUF slice. The half-swap crosses
                    partitions, so it runs as a PE matmul against a constant
                    128x128 half-rotation permutation matrix."""
                    cpy = rtmp.tile([128, 512], F16, tag="ropecpy")
                    nc.vector.tensor_scalar_mul(cpy[:, :width], ps, c1024)
                    sw_ps = popsum.tile([128, 512], F32, tag="opsum")
                    nc.tensor.matmul(
                        sw_ps[:, :width], pswap_sb, cpy[:, :width],
                        start=True, stop=True,
                    )
                    sw = rtmp.tile([128, 512], F16, tag="ropesw")
                    nc.scalar.copy(sw[:, :width], sw_ps[:, :width])
                    tmp2 = rtmp.tile([128, 512], F16, tag="ropecos")
                    nc.vector.tensor_mul(
                        tmp2[:, :width], cpy[:, :width], cos_sb[:, nsl]
                    )
                    nc.vector.tensor_mul(
                        sw[:, :width], sw[:, :width], sin_sb[:, nsl]
                    )
                    nc.vector.tensor_add(dst, tmp2[:, :width],
                                         sw[:, :width])

                k_ps = pproj.tile([128, 512], F32, tag="proj")
                v_ps = popsum.tile([128, 512], F32, tag="opsum")
                q0_ps = pwork.tile([128, 512], F32, tag="work")
                q1_ps = pwork.tile([128, 512], F32, tag="work")
                proj3([
                    (k_ps,
                     lambda s, dp: wk_sb[:, s, 2 * dp:2 * dp + 2, :]),
                    (v_ps,
                     lambda s, dp: wv_sb[:, s, 2 * dp:2 * dp + 2, :]),
                    (q0_ps,
                     lambda s, dp: wq_sb[:, 0, s, 2 * dp:2 * dp + 2, :]),
                    (q1_ps,
                     lambda s, dp: wq_sb[:, 1, s, 2 * dp:2 * dp + 2, :]),
                ])
                rope_into(k_ps, kT[:, nsl])
                rope_into(q0_ps, qT[:, 0, nsl])
                rope_into(q1_ps, qT[:, 1, nsl])

                vt_sb = small.tile([128, 512], BF16, tag="vt")
                nc.scalar.mul(vt_sb, v_ps, INV_HW)
                for jj in range(4):
                    tps = pwork.tile([128, 128], BF16, tag="work")
                    nc.tensor.transpose(
                        tps, vt_sb[:, jj * 128:(jj + 1) * 128], identbf
                    )
                    nc.vector.tensor_copy(vA[:, 4 * J + jj, 0:128], tps)

                # q2 and q3 in different pools so q3's matmuls don't wait
                # on q2's rope drain
                for t, pool in ((2, pproj), (3, pwork)):
                    q_ps = pool.tile([128, 512], F32,
                                     tag="proj" if pool is pproj
                                     else "work")
                    proj3([(
                        q_ps,
                        lambda s, dp, t=t:
                            wq_sb[:, t, s, 2 * dp:2 * dp + 2, :],
                    )])
                    rope_into(q_ps, qT[:, t, nsl])

                # prefetch next pass's hT slice; the spare hpool slot lets
                # the first tile load while this pass still computes
                if J + 1 < NI:
                    nxt = []
                    for dq in range(8):
                        t_h = hpool.tile([128, 2, 4, 512], F8, tag="ht",
                                         name=f"ht_{J + 1}_{dq}")
                        nxt.append(t_h)
                        ht_dma(nxt, J + 1, dq)
                    hts_by_chunk[J + 1] = nxt

            def do_attention(I):
                """Flattened (head, j) attention loop for q-chunk I, so S
                matmuls pipeline across head boundaries."""
                njt = 4 * I + 4   # k-tiles in causal range of this chunk
                state = {}

                def start_head(t, I=I):
                    o_ps = [
                        popsum.tile([128, 132], F32, tag="opsum",
                                    name=f"o_ps_{I}_{t}_{il}")
                        for il in range(4)
                    ]
                    return {"o_ps": o_ps, "oscs": [None] * 4}

                def finalize_il(t, il):
                    # softmax divide, emitted as soon as this query tile's
                    # PV accumulation closes: frees the PSUM accumulator
                    # early so the next head's PV can start. The 1/16 ones
                    # column makes recip = 16/den, so osc = attn x16 --
                    # pre-scaled for the fp8 split.
                    o_ps = state[t]["o_ps"]
                    recip = small.tile([128, 1], F32, tag="recip")
                    nc.vector.reciprocal(recip, o_ps[il][:, 128:129])
                    osc = small.tile([128, 128], F16, tag="osc")
                    nc.vector.tensor_scalar_mul(
                        osc, o_ps[il][:, 0:128], recip
                    )
                    state[t]["oscs"][il] = osc

                def exp_part(t, j, s_ps, I=I):
                    # issued right after the score matmul so the Act
                    # engine overlaps the following S matmuls
                    m = j - 4 * I
                    q_off = 128 * m if m > 0 else 0
                    pt = ptpool.tile([128, 512], BF16, tag="pt")
                    nc.scalar.activation(
                        pt[:, q_off:512], s_ps[:, q_off:512],
                        mybir.ActivationFunctionType.Exp,
                        scale=SCALE,
                    )
                    if m >= 0:
                        # only the boundary 128-col slice is partially
                        # masked; slices below q_off are never read by PV
                        nc.vector.tensor_mul(
                            pt[:, q_off:q_off + 128],
                            pt[:, q_off:q_off + 128],
                            masks[:, 0, 0:128],
                        )
                    return pt

                def pv_part(t, jprev, pt, I=I):
                    o_ps = state[t]["o_ps"]
                    for il in range(4):
                        i_abs = 4 * I + il
                        if jprev <= i_abs:
                            nc.tensor.matmul(
                                o_ps[il][:, 0:129],
                                pt[:, il * 128:(il + 1) * 128],
                                vA[:, jprev, 0:129],
                                start=(jprev == 0),
                                stop=(jprev == i_abs),
                            )
                            if jprev == i_abs:
                                finalize_il(t, il)

                def head_epilogue(t, I=I):
                    # transpose the f16 osc (= attn x16), then split into
                    # fp8 hi/lo on DVE for the DoubleRow o-projection
                    for il in range(4):
                        i_abs = 4 * I + il
                        csl = slice(i_abs * 128, (i_abs + 1) * 128)
                        tps = popsum.tile([128, 132], F16, tag="opsum",
                                          name=f"tps_{I}_{t}_{il}")
                        nc.tensor.transpose(
                            tps[:, 0:128], state[t]["oscs"][il], ident16
                        )
                        a8 = attnT8p[t // 2]
                        nc.vector.tensor_copy(a8[:, 0, t % 2, csl],
                                              tps[:, 0:128])
                        dtmp = small.tile([128, 128], F16, tag="dtmp")
                        nc.vector.tensor_sub(
                            dtmp, tps[:, 0:128], a8[:, 0, t % 2, csl]
                        )
                        nc.vector.tensor_copy(a8[:, 1, t % 2, csl], dtmp)

                # exp issues immediately after each score matmul (Act
                # overlaps the next S matmuls); the exp-dependent PV
                # trails by 2 steps so pt is ready when the PE needs it.
                # The previous chunk's o-projection blocks (pure PE work,
                # no Act dependency) interleave one per j-step to fill
                # the PE idle left by the Act-bound exp pipeline.
                pending = []

                def pop_pv():
                    tp_, jp_, pt_ = pending.pop(0)
                    pv_part(tp_, jp_, pt_)
                    if jp_ == njt - 1:
                        head_epilogue(tp_)

                for t in range(QH):
                    state[t] = start_head(t)
                    for j in range(njt):
                        mj = j - 4 * I
                        q_off = 128 * mj if mj > 0 else 0
                        s_ps = pwork.tile([128, 512], F32, tag="work")
                        nc.tensor.matmul(
                            s_ps[:, q_off:512],
                            kT[:, j * 128:(j + 1) * 128],
                            qT[:, t, I * 512 + q_off:(I + 1) * 512],
                            start=True, stop=True,
                        )
                        pending.append((t, j, exp_part(t, j, s_ps)))
                        if len(pending) > 2:
                            pop_pv()
                while pending:
                    pop_pv()

            def do_oproj(I):
                # ---- output projection for the 4 token tiles ----------
                # fp8 DoubleRow over the 4-head contraction (2 pairs x 3
                # compensation terms); psum holds 1024x the result.
                for il in range(4):
                    i_abs = 4 * I + il
                    csl = slice(i_abs * 128, (i_abs + 1) * 128)
                    for qtr in range(4):
                        orow = orowp.tile([128, 1024], F16, tag="orow")
                        for mc in range(2):
                            mq = qtr * 1024 + mc * 512
                            op_ps = pwork.tile([128, 512], F32,
                                               tag="work")
                            for pq in range(2):
                                oc = 2 * pq
                                aH = attnT8p[pq][:, 0, 0:2, csl]
                                aL = attnT8p[pq][:, 1, 0:2, csl]
                                wH = wo_sb[:, 0, oc:oc + 2, mq:mq + 512]
                                wL = wo_sb[:, 1, oc:oc + 2, mq:mq + 512]
                                nc.tensor.matmul(op_ps, aH, wH,
                                                 start=(pq == 0),
                                                 stop=False, perf_mode=DR)
                                nc.tensor.matmul(op_ps, aL, wH,
                                                 start=False, stop=False,
                                                 perf_mode=DR)
                                nc.tensor.matmul(op_ps, aH, wL,
                                                 start=False,
                                                 stop=(pq == 1),
                                                 perf_mode=DR)
                            # Act is idle during the o-projection; keep
                            # the psum drains off the busier DVE
                            nc.scalar.mul(
                                orow[:, mc * 512:(mc + 1) * 512], op_ps,
                                INV_HW,
                            )
                        nc.sync.dma_start(
                            out=out[i_abs * 128:(i_abs + 1) * 128,
                                    qtr * 1024:(qtr + 1) * 1024],
                            in_=orow,
                        )

            # Phase-pipelined schedule: chunk I+1's projections (pure PE
            # work) run between attention(I) and o-proj(I), so the PE
            # chews through the Act exp backlog instead of stalling on
            # the attnT8 epilogue chain.
            do_proj(0)
            for I in range(NI):
                do_attention(I)
                if I + 1 < NI:
                    do_proj(I + 1)
                do_oproj(I)
    nc.finalize()
    return nc


def _pack_inputs(h, position_ids, wq, wk, wv, wo):
    """Host-side shard + transpose + fp8 hi/lo split. Returns per-core
    input maps."""
    import ml_dtypes
    E4 = ml_dtypes.float8_e4m3

    def split8(x):
        x = np.asarray(x, dtype=np.float32)
        hi = x.astype(E4)
        lo = (x - hi.astype(np.float32)).astype(E4)
        return np.stack([hi, lo], axis=0)          # [2, ...]

    # h.T partition-major: [128, ND, S], scaled x16, fp8 hi/lo
    hT = np.ascontiguousarray(
        h.T.reshape(ND, 128, S).transpose(1, 0, 2)
    ) * np.float32(H_S)
    ht8 = np.ascontiguousarray(
        split8(hT).transpose(1, 0, 2, 3)           # [128, 2, ND, S]
    )

    # RoPE tables in transposed orientation, halves duplicated / sign-folded.
    inv = 1.0 / (1e6 ** (np.arange(0, HD, 2, dtype=np.float64) / HD))
    fr = position_ids.astype(np.float64)[None, :] * inv[:, None]   # [64, S]
    cosT = np.cos(fr).astype(np.float16)
    sinT = np.sin(fr).astype(np.float16)
    cosd = np.concatenate([cosT, cosT], axis=0)                    # [128, S]
    sind = np.concatenate([-sinT, sinT], axis=0)
    psw = np.zeros((128, 128), dtype=np.float16)
    psw[(np.arange(128) + 64) % 128, np.arange(128)] = 1.0
    iden16 = np.eye(128, dtype=np.float16)
    idenbf = np.eye(128).astype(ml_dtypes.bfloat16)
    p_i = np.arange(128)[:, None]
    f_i = np.arange(512)[None, :]
    maskd = np.stack(
        [(f_i - p_i - 128 * m >= 0) for m in range(4)], axis=1
    ).astype(ml_dtypes.bfloat16)

    in_maps = []
    for c in range(NCORES):
        wq_c = wq[c * 512:(c + 1) * 512, :]          # [512, HID]
        wk_c = wk[c * 128:(c + 1) * 128, :]
        wv_c = wv[c * 128:(c + 1) * 128, :]
        wo_c = wo[:, c * 512:(c + 1) * 512]          # [HID, 512]

        # wq: [HID, 512] -> [128, QH, 2, ND, 128] (head-major for staged DMA)
        wq_t = np.ascontiguousarray(
            wq_c.T.reshape(ND, 128, QH, 128).transpose(1, 2, 0, 3)
        ) * np.float32(W_S)                          # [128, QH, ND, 128]
        wq8 = np.ascontiguousarray(
            split8(wq_t).transpose(1, 2, 0, 3, 4)    # [128, QH, 2, ND, 128]
        )
        wk_t = np.ascontiguousarray(
            wk_c.T.reshape(ND, 128, 128).transpose(1, 0, 2)
        ) * np.float32(W_S)
        wk8 = np.ascontiguousarray(split8(wk_t).transpose(1, 0, 2, 3))
        wv_t = np.ascontiguousarray(
            wv_c.T.reshape(ND, 128, 128).transpose(1, 0, 2)
        ) * np.float32(W_S)
        wv8 = np.ascontiguousarray(split8(wv_t).transpose(1, 0, 2, 3))
        # wo: [HID, 512] -> wo_c.T [512, HID] -> [128, 2, QH, HID]
        wo_t = np.ascontiguousarray(
            wo_c.T.reshape(QH, 128, HID).transpose(1, 0, 2)
        ) * np.float32(W_S)
        wo8 = np.ascontiguousarray(split8(wo_t).transpose(1, 0, 2, 3))

        in_maps.append({
            "ht": ht8,
            "wq": wq8,
            "wk": wk8,
            "wv": wv8,
            "wo": wo8,
            "cosd": cosd,
            "sind": sind,
            "pswap": psw,
            "identd": iden16,
            "identb": idenbf,
            "maskd": maskd,
        })
    return in_maps


def kernel(h, position_ids, wq, wk, wv, wo):
    global LAST_RESULTS
    from concourse.bass_utils import run_bass_kernel_spmd

    if "nc" not in _CACHE:
        _CACHE["nc"] = _build_program()
    nc = _CACHE["nc"]

    in_maps = _pack_inputs(
        np.asarray(h, dtype=np.float32),
        np.asarray(position_ids),
        np.asarray(wq, dtype=np.float32),
        np.asarray(wk, dtype=np.float32),
        np.asarray(wv, dtype=np.float32),
        np.asarray(wo, dtype=np.float32),
    )

    trace = bool(int(os.environ.get("KERNEL_TRACE", "0")))
    res = run_bass_kernel_spmd(
        nc, in_maps, core_ids=list(range(NCORES)), trace=trace
    )
    LAST_RESULTS = res

    acc = np.zeros((S, HID), dtype=np.float32)
    for r in res.results:
        acc += r["out"].astype(np.float32)
    return acc
